# revision 7
# baseline (speedup 1.0000x reference)
"""Trainium2 Bass kernel for nn_DARPDecoder (sparse_attention).

Strategy (pure data-parallel over batch, 8 cores x 128 batches):
  score[b,n] = emb[b,n,:] . qk[b] / sqrt(D) - travel[b,n]*c ; tanh-clip, mask,
  log_softmax.  qk[b] = W_key^T q[b] eliminates the [B,N,D] K intermediate.

The 16MB bf16 embedding shard is shipped ONCE per core in its natural
[b, n, d] layout (host does only an f32->bf16 cast); both device passes
derive their tile layouts from it:
  pass 1 (sums): natural [n, d] tiles DMA'd directly (contiguous runs);
    graph/visited sums via accumulating matmuls with per-batch zero-padded
    [128,32] stationaries + tile_position, one PSUM row pair per batch.
  pass 2 (score): [d, n] tiles produced by the DMA crossbar transpose
    (dma_start_transpose) straight from the same DRAM bytes; per-batch
    score matmuls with zero-padded qk columns accumulate into one
    [128,512] PSUM tile (batch -> partition), the softmax layout.
Gathers (h_cur/h_first rows, cur_h3, travel-time rows) are indirect DMAs
against flat views of the same DRAM bytes.  Travel lookup
T[cur_h3[b], h3[b,n]] via gpsimd indirect_copy, 8 batches per call (one per
16-partition group), with host pre-wrapped h3 index layout and a
selection-matmul replicating each batch's travel-time row across its group.

All per-core inputs are packed into a single u8 blob (device reads each
field via offset-slice + bitcast APs): the axon tunnel charges ~75ms of
latency per device_put'd array, so 2 arrays beat 25.
"""

import functools
import math
import os

import numpy as np
import ml_dtypes

import jax

# Persist compiled executables (incl. the wrapped NEFF) across processes so a
# fresh process pays ~0.3s instead of a full neuronx compile. The 5s floor
# keeps small CPU jits (e.g. a reference model in the same process) out of
# the cache.
try:
    jax.config.update("jax_compilation_cache_dir",
                      os.path.expanduser("~/.jax_comp_cache"))
    jax.config.update("jax_persistent_cache_min_compile_time_secs", 5)
except Exception:
    pass

import concourse.bass as bass
import concourse.mybir as mybir
import concourse.tile as tile
from concourse import bacc
from concourse.bass_utils import run_bass_kernel_spmd

BF16 = mybir.dt.bfloat16
F32 = mybir.dt.float32
I32 = mybir.dt.int32
U16 = mybir.dt.uint16
U8 = mybir.dt.uint8
FP8 = mybir.dt.float8e4
Alu = mybir.AluOpType
AF = mybir.ActivationFunctionType
AX = mybir.AxisListType

B, N, D, NCORES = 1024, 512, 128, 8
BC = B // NCORES  # 128 batches/core
NCH, CHB = 16, 8  # 16 stream chunks x 8 batches
MAX_TIME = 1440.0
TANH_CLIP = 10.0
C_TRAVEL = 1.0 / MAX_TIME / math.sqrt(2.0)
INV_SQRT_D = 1.0 / math.sqrt(D)
NBF = np.dtype(ml_dtypes.bfloat16)
DEBUG_TAPS = False

# ---------------------------------------------------------------------------
# blob layout: every per-core input at a fixed 128B-aligned byte offset
_FIELDS = [
    ("emb", BF16, (BC, N, D)),
    ("ttm", BF16, (N, N)),
    ("h3u", U16, (BC * N, 1)),
    ("h3_wrap", U16, (NCH, 128, N // 16)),
    ("visT_bf", BF16, (N, BC)),
    ("s_sel", FP8, (NCH, 128, 128)),
    ("ident", BF16, (128, 128)),
    ("p_a", BF16, (128, 128)),
    ("p_b", BF16, (128, 128)),
    ("w_last", BF16, (D, D)),
    ("w_first", BF16, (D, D)),
    ("w_graph", BF16, (D, D)),
    ("w_visited", BF16, (D, D)),
    ("w_keyT", BF16, (D, D)),
    ("w_state", BF16, (3, D)),
    ("b_state", F32, (D, 1)),
    ("scal4", F32, (BC, 4)),
    ("c_half", F32, (128, 1)),
    ("iota512f", F32, (BC, 1)),
    ("cur_i", I32, (BC, 1)),
    ("prev_i", I32, (BC, 1)),
    ("first_i", I32, (BC, 1)),
    ("vis_rows", U8, (BC, N)),
    ("am_rows", U8, (BC, N)),
]


def _layout():
    offs = {}
    off = 0
    for name, dt_, shape in _FIELDS:
        esz = mybir.dt.size(dt_)
        nb = esz * int(np.prod(shape))
        offs[name] = (off, dt_, shape, nb)
        off += (nb + 127) // 128 * 128
    return offs, (off + 1023) // 1024 * 1024


_OFFS, BLOB_BYTES = _layout()
_REARR = {2: "(a b) -> a b", 3: "(a b c) -> a b c"}


def _emit(nc, tc, T):
    """Emit the whole per-core program. T: dict of dram tensor handles."""
    blob = T["blob"].ap()
    ap = {k: v.ap() for k, v in T.items() if k != "blob"}

    def bap(name):
        off, dt_, shape, nb = _OFFS[name]
        a = blob[off : off + nb].bitcast(dt_)
        kw = dict(zip("abc", shape))
        return a.rearrange(_REARR[len(shape)], **kw)

    emb_flat = bap("emb").rearrange("b n d -> (b n) d")  # [BC*N, D]

    with (
        tc.tile_pool(name="cp", bufs=1) as cp,
        tc.tile_pool(name="st", bufs=3) as st,
        tc.tile_pool(name="wk", bufs=2) as wk,
        tc.tile_pool(name="ps_sum", bufs=1, space="PSUM") as ps_sum,
        tc.tile_pool(name="ps_sm", bufs=1, space="PSUM") as ps_sm,
        tc.tile_pool(name="ps_tr", bufs=2, space="PSUM") as ps_tr,
        tc.tile_pool(name="ps_sc", bufs=1, space="PSUM") as ps_sc,
    ):
        def load(name, shape, dtype, src_ap=None, tag=None):
            t = cp.tile(shape, dtype, name=name, tag=tag or name)
            nc.sync.dma_start(out=t[:], in_=src_ap if src_ap is not None else bap(name))
            return t

        # ---------- small loads ----------
        wl = load("w_last", [D, D], BF16)
        wf = load("w_first", [D, D], BF16)
        wg = load("w_graph", [D, D], BF16)
        wv = load("w_visited", [D, D], BF16)
        wkT = load("w_keyT", [D, D], BF16)
        ws = load("w_state", [3, D], BF16)
        bst = load("b_state", [D, 1], F32)
        sc4 = load("scal4", [BC, 4], F32)
        cur = load("cur_i", [BC, 1], I32)
        prv = load("prev_i", [BC, 1], I32)
        fst = load("first_i", [BC, 1], I32)
        io5 = load("iota512f", [BC, 1], F32)
        idn = load("ident", [128, 128], BF16)
        pa = load("p_a", [128, 128], BF16)
        pb = load("p_b", [128, 128], BF16)
        chf = load("c_half", [128, 1], F32)
        vis_u8 = load("vis_rows", [BC, N], U8)
        am_u8 = load("am_rows", [BC, N], U8)
        ssel = [load(f"ssel{k}", [128, 128], FP8, bap("s_sel")[k]) for k in range(16)]
        h3w = [load(f"h3w{k}", [128, N // 16], U16, bap("h3_wrap")[k]) for k in range(16)]
        visT = [load(f"visT{t}", [128, BC], BF16, bap("visT_bf")[128 * t : 128 * (t + 1), :]) for t in range(4)]

        # ---------- masks / counts ----------
        visf = cp.tile([BC, N], F32, name="visf")
        nc.vector.tensor_copy(out=visf[:], in_=vis_u8[:])
        amf = cp.tile([BC, N], F32, name="amf")
        nc.vector.tensor_copy(out=amf[:], in_=am_u8[:])
        vc = cp.tile([BC, 1], F32, name="vc")
        nc.vector.tensor_reduce(out=vc[:], in_=visf[:], axis=AX.X, op=Alu.add)
        nc.vector.tensor_scalar_max(out=vc[:], in0=vc[:], scalar1=1.0)
        vcr = cp.tile([BC, 1], F32, name="vcr")
        nc.vector.reciprocal(out=vcr[:], in_=vc[:])
        vcrb = cp.tile([BC, 1], BF16, name="vcrb")
        nc.vector.tensor_copy(out=vcrb[:], in_=vcr[:])
        # per-PSUM-row descale vector: even rows 1/512 (graph mean), odd rows 1/vcount
        vcrp = []
        for half, P in ((0, pa), (1, pb)):
            pm = ps_sm.tile([128, 1], F32, tag="sm")
            nc.tensor.matmul(out=pm[:], lhsT=P[:], rhs=vcrb[:], start=True, stop=True)
            vp = cp.tile([128, 1], F32, name=f"vcrp{half}", tag=f"vcrp{half}")
            nc.vector.tensor_add(out=vp[:], in0=pm[:], in1=chf[:])
            vcrp.append(vp)

        # ---------- first-node bookkeeping + gather indices (f32 exact) ----------
        curf = cp.tile([BC, 1], F32, name="curf")
        nc.vector.tensor_copy(out=curf[:], in_=cur[:])
        prvf = cp.tile([BC, 1], F32, name="prvf")
        nc.vector.tensor_copy(out=prvf[:], in_=prv[:])
        fstf = cp.tile([BC, 1], F32, name="fstf")
        nc.vector.tensor_copy(out=fstf[:], in_=fst[:])
        t1 = cp.tile([BC, 1], F32, name="t1")
        nc.vector.tensor_single_scalar(out=t1[:], in_=prvf[:], scalar=0.0, op=Alu.is_equal)
        t2 = cp.tile([BC, 1], F32, name="t2")
        nc.vector.tensor_single_scalar(out=t2[:], in_=curf[:], scalar=0.0, op=Alu.not_equal)
        ld = cp.tile([BC, 1], F32, name="ld")
        nc.vector.tensor_mul(out=ld[:], in0=t1[:], in1=t2[:])
        dd = cp.tile([BC, 1], F32, name="dd")
        nc.vector.tensor_sub(out=dd[:], in0=curf[:], in1=fstf[:])
        nc.vector.tensor_mul(out=dd[:], in0=ld[:], in1=dd[:])
        fnf = cp.tile([BC, 1], F32, name="fnf")
        nc.vector.tensor_add(out=fnf[:], in0=fstf[:], in1=dd[:])
        nc.vector.tensor_mul(out=fnf[:], in0=fnf[:], in1=t2[:])

        gcf = cp.tile([BC, 1], F32, name="gcf")
        nc.vector.tensor_add(out=gcf[:], in0=io5[:], in1=curf[:])
        gcur = cp.tile([BC, 1], I32, name="gcur")
        nc.vector.tensor_copy(out=gcur[:], in_=gcf[:])
        gff = cp.tile([BC, 1], F32, name="gff")
        nc.vector.tensor_add(out=gff[:], in0=io5[:], in1=fnf[:])
        gfn = cp.tile([BC, 1], I32, name="gfn")
        nc.vector.tensor_copy(out=gfn[:], in_=gff[:])

        # ---------- gathers ----------
        hc_rows = cp.tile([BC, D], BF16, name="hc_rows")
        nc.gpsimd.indirect_dma_start(
            out=hc_rows[:], out_offset=None, in_=emb_flat,
            in_offset=bass.IndirectOffsetOnAxis(ap=gcur[:, :1], axis=0))
        hf_rows = cp.tile([BC, D], BF16, name="hf_rows")
        nc.gpsimd.indirect_dma_start(
            out=hf_rows[:], out_offset=None, in_=emb_flat,
            in_offset=bass.IndirectOffsetOnAxis(ap=gfn[:, :1], axis=0))
        # indirect DMA sources must sit at offset 0: view the whole blob as
        # rows of the target width and fold the field's base row into the
        # index (all values < 2^24, exact in f32).
        h3_rows = blob.bitcast(U16).rearrange("(r c) -> r c", c=1)
        h3_base = float(_OFFS["h3u"][0] // 2)
        gh3f = cp.tile([BC, 1], F32, name="gh3f")
        nc.vector.tensor_scalar_add(out=gh3f[:], in0=gcf[:], scalar1=h3_base)
        gh3 = cp.tile([BC, 1], I32, name="gh3")
        nc.vector.tensor_copy(out=gh3[:], in_=gh3f[:])
        ch3u = cp.tile([BC, 1], U16, name="ch3u")
        nc.gpsimd.indirect_dma_start(
            out=ch3u[:], out_offset=None, in_=h3_rows,
            in_offset=bass.IndirectOffsetOnAxis(ap=gh3[:, :1], axis=0))
        ttm_rows = blob.bitcast(BF16).rearrange("(r c) -> r c", c=N)
        ttm_base = float(_OFFS["ttm"][0] // (2 * N))
        gtt = cp.tile([BC, 1], F32, name="gtt")
        nc.vector.tensor_copy(out=gtt[:], in_=ch3u[:])
        nc.vector.tensor_scalar_add(out=gtt[:], in0=gtt[:], scalar1=ttm_base)
        ch3 = cp.tile([BC, 1], I32, name="ch3")
        nc.vector.tensor_copy(out=ch3[:], in_=gtt[:])
        rbf0 = cp.tile([BC, N], BF16, name="rbf0")
        nc.gpsimd.indirect_dma_start(
            out=rbf0[:], out_offset=None, in_=ttm_rows,
            in_offset=bass.IndirectOffsetOnAxis(ap=ch3[:, :1], axis=0))
        rbf = cp.tile([BC, N], BF16, name="rbf")  # pre-scaled by C_TRAVEL
        nc.vector.tensor_scalar_mul(out=rbf[:], in0=rbf0[:], scalar1=C_TRAVEL)

        # ---------- travel: T[cur_h3[b], h3[b,:]] * C_TRAVEL, 8 batches/call ----------
        trav = cp.tile([BC, N], F32, name="trav")
        for k in range(16):
            pt = ps_tr.tile([128, N], F32, tag="trep")
            nc.tensor.matmul(out=pt[:], lhsT=ssel[k][:], rhs=rbf[:], start=True, stop=True)
            sck = wk.tile([128, N], F32, tag="travsc")
            nc.vector.tensor_copy(out=sck[:], in_=pt[:])
            gk = wk.tile([128, N], F32, tag="travg")
            nc.gpsimd.indirect_copy(out=gk[:], data=sck[:], idxs=h3w[k][:],
                                    i_know_ap_gather_is_preferred=True)
            for g in range(8):  # extract batch row 16g+k (partition strides are
                r0 = 16 * g + k  # not AP-expressible; 8 tiny row DMAs instead)
                nc.sync.dma_start(out=trav[r0 : r0 + 1, :], in_=gk[r0 : r0 + 1, :])

        # ---------- W2 stationaries for the sums pass ----------
        # per n-tile t: [128, 4096]; batch b owns cols [32b, 32b+32):
        #   col 32b+2s   = 1.0   (s = b%16)  -> graph sum row
        #   col 32b+2s+1 = vf_b  -> visited sum row
        w2 = []
        for t in range(4):
            w = cp.tile([128, 4096], BF16, name=f"w2_{t}", tag=f"w2_{t}")
            nc.vector.memset(w[:], 0.0)
            ones_ap = w[:].rearrange("p (u c) -> p u c", u=8)[:, :, 0:512:34]
            nc.vector.memset(ones_ap, 1.0)
            vf_ap = w[:].rearrange("p (u c) -> p u c", u=8)[:, :, 1:512:34]
            nc.vector.tensor_copy(out=vf_ap, in_=visT[t][:].rearrange("p (u s) -> p u s", u=8))
            w2.append(w)

        # ---------- pass 1: natural-layout stream -> graph/visited sums ----------
        # chunk k: [128, (j t d)] with partition p = n % 128, t = n // 128,
        # j = batch within chunk; DMA'd straight from the [b, n, d] DRAM bytes.
        psA = ps_sum.tile([128, D], F32, tag="sumA")
        psB = ps_sum.tile([128, D], F32, tag="sumB")
        for k in range(NCH):
            nat = st.tile([128, 4096], BF16, tag="nat")
            src = bap("emb")[CHB * k : CHB * (k + 1)].rearrange("j (t p) d -> p j t d", p=128)
            nc.sync.dma_start(out=nat[:].rearrange("p (j t d) -> p j t d", j=CHB, t=4), in_=src)
            for j in range(CHB):
                b = k * CHB + j
                half, r = b // 64, b % 64
                jj, s = r // 16, r % 16
                ps = psA if half == 0 else psB
                for t in range(4):
                    nc.tensor.matmul(
                        out=ps[32 * jj : 32 * jj + 32, :],
                        lhsT=w2[t][:, 32 * b : 32 * b + 32],
                        rhs=nat[:, (j * 4 + t) * 128 : (j * 4 + t + 1) * 128],
                        start=(s == 0 and t == 0), stop=True,
                        tile_position=(0, 32 * jj), skip_group_check=True)

        # ---------- sums -> G^T / V^T (dense, bf16, [128e, 128b]) ----------
        gt = cp.tile([128, BC], BF16, name="gt")
        vt = cp.tile([128, BC], BF16, name="vt")
        for half, ps in ((0, psA), (1, psB)):
            gvr = wk.tile([128, 128], BF16, tag="gvr")
            nc.vector.tensor_scalar(out=gvr[:], in0=ps[:], scalar1=vcrp[half][:, :1],
                                    scalar2=None, op0=Alu.mult)
            pt = ps_tr.tile([128, 128], BF16, tag="gvt", bufs=1)
            nc.tensor.transpose(out=pt[:], in_=gvr[:], identity=idn[:])
            # cols m=32j+2s -> batch 64*half+16j+s
            src_g = pt[:].rearrange("p (j c) -> p j c", j=4)[:, :, 0:32:2]
            src_v = pt[:].rearrange("p (j c) -> p j c", j=4)[:, :, 1:32:2]
            dst_g = gt[:, 64 * half : 64 * half + 64].rearrange("p (j s) -> p j s", j=4)
            dst_v = vt[:, 64 * half : 64 * half + 64].rearrange("p (j s) -> p j s", j=4)
            nc.vector.tensor_copy(out=dst_g, in_=src_g)
            nc.vector.tensor_copy(out=dst_v, in_=src_v)

        # ---------- h_cur/h_first transposes ----------
        hct = cp.tile([128, BC], BF16, name="hct")
        pt1 = ps_tr.tile([128, 128], BF16, tag="gvt", bufs=1)
        nc.tensor.transpose(out=pt1[:], in_=hc_rows[:], identity=idn[:])
        nc.vector.tensor_copy(out=hct[:], in_=pt1[:])
        hft = cp.tile([128, BC], BF16, name="hft")
        pt2 = ps_tr.tile([128, 128], BF16, tag="gvt", bufs=1)
        nc.tensor.transpose(out=pt2[:], in_=hf_rows[:], identity=idn[:])
        nc.vector.tensor_copy(out=hft[:], in_=pt2[:])

        # ---------- state feats ----------
        sf = cp.tile([BC, 3], F32, name="sf")
        nc.vector.tensor_sub(out=sf[:, 0:1], in0=sc4[:, 2:3], in1=sc4[:, 1:2])
        nc.vector.tensor_scalar_mul(out=sf[:, 1:2], in0=sc4[:, 0:1], scalar1=1.0 / MAX_TIME)
        nc.vector.tensor_scalar_mul(out=sf[:, 2:3], in0=sc4[:, 3:4], scalar1=1.0 / (2.0 * N))
        sfb = cp.tile([BC, 3], BF16, name="sfb")
        nc.vector.tensor_copy(out=sfb[:], in_=sf[:])
        pt3 = ps_tr.tile([128, 128], BF16, tag="gvt", bufs=1)
        nc.tensor.transpose(out=pt3[:3, :], in_=sfb[:], identity=idn[:])
        sft = cp.tile([3, BC], BF16, name="sft")
        nc.vector.tensor_copy(out=sft[:], in_=pt3[:3, :BC])

        # ---------- q^T and qk^T ----------
        psq = ps_sm.tile([128, BC], F32, tag="sm")
        nc.tensor.matmul(out=psq[:], lhsT=wl[:], rhs=hct[:], start=True, stop=True)
        nc.tensor.matmul(out=psq[:], lhsT=wf[:], rhs=hft[:], start=False, stop=True,
                         skip_group_check=True)
        nc.tensor.matmul(out=psq[:], lhsT=wg[:], rhs=gt[:], start=False, stop=True,
                         skip_group_check=True)
        nc.tensor.matmul(out=psq[:], lhsT=wv[:], rhs=vt[:], start=False, stop=True,
                         skip_group_check=True)
        nc.tensor.matmul(out=psq[:], lhsT=ws[:], rhs=sft[:], start=False, stop=True,
                         skip_group_check=True)
        qt = cp.tile([128, BC], BF16, name="qt")
        nc.vector.tensor_scalar(out=qt[:], in0=psq[:], scalar1=bst[:, :1], scalar2=None,
                                op0=Alu.add)
        psk = ps_sm.tile([128, BC], F32, tag="sm")
        nc.tensor.matmul(out=psk[:], lhsT=wkT[:], rhs=qt[:], start=True, stop=True)
        qk = cp.tile([128, BC], BF16, name="qk")
        nc.vector.tensor_scalar_mul(out=qk[:], in0=psk[:], scalar1=INV_SQRT_D)

        # ---------- qkpad: batch b -> col 32b + (b%32) ----------
        qkp = cp.tile([128, 4096], BF16, name="qkp")
        nc.vector.memset(qkp[:], 0.0)
        for J in range(4):
            # batch b = 32J + r -> col 32b + r = 1024J + 33r (out row = 32J + r = b)
            nc.vector.tensor_copy(out=qkp[:, 1024 * J : 1024 * (J + 1) : 33],
                                  in_=qk[:, 32 * J : 32 * (J + 1)])

        # ---------- pass 2: crossbar-transposed stream -> score psum [128b, 512n] ----------
        pssc = ps_sc.tile([128, N], F32, tag="score")
        for k in range(NCH):
            et = st.tile([128, 4096], BF16, tag="et")
            nc.sync.dma_start_transpose(et[:], emb_flat[4096 * k : 4096 * (k + 1), :])
            for j in range(CHB):
                b = k * CHB + j
                J, r = b // 32, b % 32
                nc.tensor.matmul(
                    out=pssc[32 * J : 32 * J + 32, :],
                    lhsT=qkp[:, 32 * b : 32 * b + 32],
                    rhs=et[:, j * N : (j + 1) * N],
                    start=(r == 0), stop=True,
                    tile_position=(0, 32 * J), skip_group_check=True)

        # ---------- epilogue: travel, tanh, mask, log_softmax ----------
        if DEBUG_TAPS:
            for nm, tl in [("d_trav", trav), ("d_gt", gt), ("d_vt", vt),
                           ("d_hct", hct), ("d_hft", hft), ("d_qt", qt), ("d_qk", qk)]:
                tmpd = wk.tile([tl.shape[0], tl.shape[1]], F32, tag=f"tap{nm}")
                nc.vector.tensor_copy(out=tmpd[:], in_=tl[:])
                nc.sync.dma_start(out=ap[nm], in_=tmpd[:])
        ssb = wk.tile([BC, N], F32, tag="ssb")
        nc.vector.tensor_sub(out=ssb[:], in0=pssc[:], in1=trav[:])
        if DEBUG_TAPS:
            nc.sync.dma_start(out=ap["d_score"], in_=ssb[:])
        th = wk.tile([BC, N], F32, tag="th")
        nc.scalar.activation(out=th[:], in_=ssb[:], func=AF.Tanh, scale=1.0 / TANH_CLIP)
        m10 = wk.tile([BC, N], F32, tag="m10")
        nc.vector.tensor_scalar_mul(out=m10[:], in0=amf[:], scalar1=TANH_CLIP)
        m2 = wk.tile([BC, N], F32, tag="m2")
        nc.vector.tensor_scalar(out=m2[:], in0=amf[:], scalar1=1.0, scalar2=1e8,
                                op0=Alu.subtract, op1=Alu.mult)
        msk = wk.tile([BC, N], F32, tag="msk")
        nc.vector.tensor_mul(out=msk[:], in0=th[:], in1=m10[:])
        nc.vector.tensor_add(out=msk[:], in0=msk[:], in1=m2[:])
        if DEBUG_TAPS:
            nc.sync.dma_start(out=ap["d_msk"], in_=msk[:])

        mx = cp.tile([BC, 1], F32, name="mx")
        nc.vector.tensor_reduce(out=mx[:], in_=msk[:], axis=AX.X, op=Alu.max)
        ngm = cp.tile([BC, 1], F32, name="ngm")
        nc.vector.tensor_scalar_mul(out=ngm[:], in0=mx[:], scalar1=-1.0)
        ex = wk.tile([BC, N], F32, tag="ex")
        se = cp.tile([BC, 1], F32, name="se")
        nc.scalar.activation(out=ex[:], in_=msk[:], func=AF.Exp, bias=ngm[:, :1],
                             scale=1.0, accum_out=se[:])
        lse = cp.tile([BC, 1], F32, name="lse")
        nc.scalar.activation(out=lse[:], in_=se[:], func=AF.Ln)
        sh = wk.tile([BC, N], F32, tag="sh")
        nc.vector.tensor_scalar(out=sh[:], in0=msk[:], scalar1=mx[:, :1], scalar2=None,
                                op0=Alu.subtract)
        fin = wk.tile([BC, N], F32, tag="fin")
        nc.vector.tensor_scalar(out=fin[:], in0=sh[:], scalar1=lse[:, :1], scalar2=None,
                                op0=Alu.subtract)
        nc.sync.dma_start(out=ap["out"], in_=fin[:])


def build_program():
    nc = bacc.Bacc("TRN2", target_bir_lowering=False, debug=False)
    dt = nc.dram_tensor
    T = {}
    T["blob"] = dt("blob", [BLOB_BYTES], U8, kind="ExternalInput")
    T["out"] = dt("out", [BC, N], F32, kind="ExternalOutput")
    if DEBUG_TAPS:
        for nm, shp in [("d_trav", [BC, N]), ("d_gt", [128, BC]), ("d_vt", [128, BC]),
                        ("d_hct", [128, BC]), ("d_hft", [128, BC]), ("d_qt", [128, BC]),
                        ("d_qk", [128, BC]), ("d_score", [BC, N]), ("d_msk", [BC, N])]:
            T[nm] = dt(nm, shp, F32, kind="ExternalOutput")

    with tile.TileContext(nc) as tc:
        _emit(nc, tc, T)
    nc.compile()
    return nc


@functools.cache
def _cached_program():
    return build_program()


def _consts():
    c = {}
    c["ident"] = np.eye(128, dtype=NBF)
    s = np.zeros((16, 128, 128), dtype=mybir.dt.np(FP8))
    pidx = np.arange(128)
    for k in range(16):
        s[k, (pidx // 16) * 16 + k, pidx] = 1.0
    c["s_sel"] = s
    pa = np.zeros((128, 128), dtype=NBF)
    pb = np.zeros((128, 128), dtype=NBF)
    for b in range(64):
        m = 32 * (b // 16) + 2 * (b % 16) + 1
        pa[b, m] = 1
        pb[64 + b, m] = 1
    c["p_a"], c["p_b"] = pa, pb
    ch = np.zeros((128, 1), np.float32)
    ch[0::2] = 1.0 / N
    c["c_half"] = ch
    c["iota512f"] = (np.arange(BC, dtype=np.float32) * N)[:, None]
    return c


def make_in_map(inputs, core, consts=None):
    """Host-side shard + pack into the per-core blob (pure layout/dtype work)."""
    sl = slice(BC * core, BC * (core + 1))
    blob = np.zeros(BLOB_BYTES, np.uint8)

    def fv(name):
        off, dt_, shape, nb = _OFFS[name]
        return blob[off : off + nb].view(mybir.dt.np(dt_)).reshape(shape)

    fv("emb")[:] = np.asarray(inputs["node_emb"][sl], dtype=np.float32)
    h3 = np.asarray(inputs["h3_indices"][sl]).astype(np.uint16)  # [128, 512]
    fv("h3u")[:] = h3.reshape(BC * N, 1)
    fv("h3_wrap")[:] = np.ascontiguousarray(
        h3.reshape(8, 16, 32, 16).transpose(1, 0, 3, 2)).reshape(16, 128, 32)
    fv("ttm")[:] = np.asarray(inputs["travel_time_matrix"], dtype=np.float32)
    vis = np.asarray(inputs["visited"][sl]).astype(np.uint8)
    fv("vis_rows")[:] = vis
    fv("visT_bf")[:] = vis.T
    fv("am_rows")[:] = np.asarray(inputs["action_mask"][sl]).astype(np.uint8)
    fv("w_last")[:] = np.asarray(inputs["W_last"], np.float32)
    fv("w_first")[:] = np.asarray(inputs["W_first"], np.float32)
    fv("w_graph")[:] = np.asarray(inputs["W_graph"], np.float32)
    fv("w_visited")[:] = np.asarray(inputs["W_visited"], np.float32)
    fv("w_keyT")[:] = np.asarray(inputs["W_key"], np.float32).T
    fv("w_state")[:] = np.asarray(inputs["W_state"], np.float32)
    fv("b_state")[:] = np.asarray(inputs["b_state"], np.float32).reshape(D, 1)
    fv("scal4")[:] = np.concatenate(
        [np.asarray(inputs["current_time"][sl], np.float32),
         np.asarray(inputs["used_capacity"][sl], np.float32),
         np.asarray(inputs["vehicle_capacity"][sl], np.float32),
         np.asarray(inputs["i"][sl]).astype(np.float32)], axis=1)
    fv("cur_i")[:] = np.asarray(inputs["current_node"][sl]).astype(np.int32).reshape(BC, 1)
    fv("prev_i")[:] = np.asarray(inputs["previous_action"][sl]).astype(np.int32).reshape(BC, 1)
    fv("first_i")[:] = np.asarray(inputs["first_node"][sl]).astype(np.int32).reshape(BC, 1)
    for k, v in (consts or _consts()).items():
        fv(k)[:] = v
    return {"blob": blob}


_last_results = None


def kernel(**inputs):
    global _last_results
    from concurrent.futures import ThreadPoolExecutor
    nc = _cached_program()
    consts = _consts()
    inputs = {k: np.asarray(v) for k, v in inputs.items()}
    with ThreadPoolExecutor(NCORES) as ex:
        in_maps = list(ex.map(lambda c: make_in_map(inputs, c, consts),
                              range(NCORES)))
    trace = bool(int(os.environ.get("KERNEL_TRACE", "0")))
    rr = run_bass_kernel_spmd(nc, in_maps, list(range(NCORES)), trace=trace)
    _last_results = rr
    out = np.concatenate([np.asarray(rr.results[c]["out"], np.float32)
                          for c in range(NCORES)], axis=0)
    return out


# revision 18
# speedup vs baseline: 1.0826x; 1.0826x over previous
"""Trainium2 Bass kernel for nn_DARPDecoder (sparse_attention).

Strategy (pure data-parallel over batch, 8 cores x 128 batches):
  score[b,n] = emb[b,n,:] . qk[b] / sqrt(D) - travel[b,n]*c ; tanh-clip, mask,
  log_softmax.  qk[b] = W_key^T q[b] eliminates the [B,N,D] K intermediate.

The 16MB bf16 embedding shard is shipped ONCE per core in its natural
[b, n, d] layout (host does only an f32->bf16 cast) and read from HBM
ONCE: a single stream of [d, n] chunks produced by the DMA crossbar
transpose (dma_start_transpose) stays fully resident in SBUF (16 x 8KB
per partition).  As each chunk lands:
  - the PE transposes its 32 [128,128] blocks back to natural layout
    (packed 8-up into one PSUM bank, copied out by DVE/Act alternately),
  - graph/visited sums accumulate via matmuls with per-batch zero-padded
    [128,32] stationaries + tile_position, one PSUM row pair per batch.
After qk is formed, per-batch score matmuls with zero-padded qk columns
re-read the resident chunks and accumulate into one [128,512] PSUM tile
(batch -> partition), the softmax layout.  HBM traffic: 16MB instead of
the two-stream 32MB; the stream phase is DMA-paced, the score phase PE-
paced.
Gathers (h_cur/h_first rows, cur_h3, travel-time rows) are indirect DMAs
against flat views of the same DRAM bytes.  Travel lookup
T[cur_h3[b], h3[b,n]] via gpsimd indirect_copy, 8 batches per call (one per
16-partition group), with host pre-wrapped h3 index layout and a
selection-matmul replicating each batch's travel-time row across its group.

All per-core inputs are packed into a single u8 blob (device reads each
field via offset-slice + bitcast APs): the axon tunnel charges ~75ms of
latency per device_put'd array, so 2 arrays beat 25.
"""

import functools
import math
import os

import numpy as np
import ml_dtypes

import jax

# Persist compiled executables (incl. the wrapped NEFF) across processes so a
# fresh process pays ~0.3s instead of a full neuronx compile. The 5s floor
# keeps small CPU jits (e.g. a reference model in the same process) out of
# the cache.
try:
    jax.config.update("jax_compilation_cache_dir",
                      os.path.expanduser("~/.jax_comp_cache"))
    jax.config.update("jax_persistent_cache_min_compile_time_secs", 5)
except Exception:
    pass

import concourse.bass as bass
import concourse.mybir as mybir
import concourse.tile as tile
from concourse import bacc
from concourse.bass_utils import run_bass_kernel_spmd

BF16 = mybir.dt.bfloat16
F32 = mybir.dt.float32
I32 = mybir.dt.int32
U16 = mybir.dt.uint16
U8 = mybir.dt.uint8
FP8 = mybir.dt.float8e4
Alu = mybir.AluOpType
AF = mybir.ActivationFunctionType
AX = mybir.AxisListType

B, N, D, NCORES = 1024, 512, 128, 8
BC = B // NCORES  # 128 batches/core
NCH, CHB = 16, 8  # 16 stream chunks x 8 batches
MAX_TIME = 1440.0
TANH_CLIP = 10.0
C_TRAVEL = 1.0 / MAX_TIME / math.sqrt(2.0)
INV_SQRT_D = 1.0 / math.sqrt(D)
NBF = np.dtype(ml_dtypes.bfloat16)
DEBUG_TAPS = False

# ---------------------------------------------------------------------------
# blob layout: every per-core input at a fixed 128B-aligned byte offset
_FIELDS = [
    ("emb", BF16, (BC, N, D)),
    ("ttm", BF16, (N, N)),
    ("h3u", U16, (BC * N, 1)),
    ("h3_wrap", U16, (NCH, 128, N // 16)),
    ("visT_bf", BF16, (N, BC)),
    ("s_sel", FP8, (NCH, 128, 128)),
    ("ident", BF16, (128, 128)),
    ("p_a", BF16, (128, 128)),
    ("p_b", BF16, (128, 128)),
    ("w_last", BF16, (D, D)),
    ("w_first", BF16, (D, D)),
    ("w_graph", BF16, (D, D)),
    ("w_visited", BF16, (D, D)),
    ("w_keyT", BF16, (D, D)),
    ("w_state", BF16, (3, D)),
    ("b_state", F32, (D, 1)),
    ("scal4", F32, (BC, 4)),
    ("c_half", F32, (128, 1)),
    ("iota512f", F32, (BC, 1)),
    ("cur_i", I32, (BC, 1)),
    ("prev_i", I32, (BC, 1)),
    ("first_i", I32, (BC, 1)),
    ("vis_rows", U8, (BC, N)),
    ("am_rows", U8, (BC, N)),
]


def _layout():
    offs = {}
    off = 0
    for name, dt_, shape in _FIELDS:
        esz = mybir.dt.size(dt_)
        nb = esz * int(np.prod(shape))
        offs[name] = (off, dt_, shape, nb)
        off += (nb + 127) // 128 * 128
    return offs, (off + 1023) // 1024 * 1024


_OFFS, BLOB_BYTES = _layout()
_REARR = {2: "(a b) -> a b", 3: "(a b c) -> a b c"}


def _emit(nc, tc, T):
    """Emit the whole per-core program. T: dict of dram tensor handles."""
    blob = T["blob"].ap()
    ap = {k: v.ap() for k, v in T.items() if k != "blob"}

    def bap(name):
        off, dt_, shape, nb = _OFFS[name]
        a = blob[off : off + nb].bitcast(dt_)
        kw = dict(zip("abc", shape))
        return a.rearrange(_REARR[len(shape)], **kw)

    emb_flat = bap("emb").rearrange("b n d -> (b n) d")  # [BC*N, D]

    with (
        tc.tile_pool(name="cp", bufs=1) as cp,
        tc.tile_pool(name="st", bufs=1) as st,
        tc.tile_pool(name="natp", bufs=2) as natp,
        tc.tile_pool(name="wk", bufs=2) as wk,
        tc.tile_pool(name="ps_sum", bufs=1, space="PSUM") as ps_sum,
        tc.tile_pool(name="ps_sm", bufs=1, space="PSUM") as ps_sm,
        tc.tile_pool(name="ps_tr", bufs=1, space="PSUM") as ps_tr,
        tc.tile_pool(name="ps_tb", bufs=2, space="PSUM") as ps_tb,
        tc.tile_pool(name="ps_sc", bufs=1, space="PSUM") as ps_sc,
    ):
        def load(name, shape, dtype, src_ap=None, tag=None):
            t = cp.tile(shape, dtype, name=name, tag=tag or name)
            nc.sync.dma_start(out=t[:], in_=src_ap if src_ap is not None else bap(name))
            return t

        # ---------- small loads ----------
        wl = load("w_last", [D, D], BF16)
        wf = load("w_first", [D, D], BF16)
        wg = load("w_graph", [D, D], BF16)
        wv = load("w_visited", [D, D], BF16)
        wkT = load("w_keyT", [D, D], BF16)
        ws = load("w_state", [3, D], BF16)
        bst = load("b_state", [D, 1], F32)
        sc4 = load("scal4", [BC, 4], F32)
        cur = load("cur_i", [BC, 1], I32)
        prv = load("prev_i", [BC, 1], I32)
        fst = load("first_i", [BC, 1], I32)
        io5 = load("iota512f", [BC, 1], F32)
        idn = load("ident", [128, 128], BF16)
        pa = load("p_a", [128, 128], BF16)
        pb = load("p_b", [128, 128], BF16)
        chf = load("c_half", [128, 1], F32)
        vis_u8 = load("vis_rows", [BC, N], U8)
        am_u8 = load("am_rows", [BC, N], U8)
        ssel = [load(f"ssel{k}", [128, 128], FP8, bap("s_sel")[k]) for k in range(16)]
        h3w = [load(f"h3w{k}", [128, N // 16], U16, bap("h3_wrap")[k]) for k in range(16)]
        visT = [load(f"visT{t}", [128, BC], BF16, bap("visT_bf")[128 * t : 128 * (t + 1), :]) for t in range(4)]

        # ---------- masks / counts ----------
        visf = cp.tile([BC, N], F32, name="visf")
        nc.vector.tensor_copy(out=visf[:], in_=vis_u8[:])
        amf = cp.tile([BC, N], F32, name="amf")
        nc.vector.tensor_copy(out=amf[:], in_=am_u8[:])
        vc = cp.tile([BC, 1], F32, name="vc")
        nc.vector.tensor_reduce(out=vc[:], in_=visf[:], axis=AX.X, op=Alu.add)
        nc.vector.tensor_scalar_max(out=vc[:], in0=vc[:], scalar1=1.0)
        vcr = cp.tile([BC, 1], F32, name="vcr")
        nc.vector.reciprocal(out=vcr[:], in_=vc[:])
        vcrb = cp.tile([BC, 1], BF16, name="vcrb")
        nc.vector.tensor_copy(out=vcrb[:], in_=vcr[:])
        # per-PSUM-row descale vector: even rows 1/512 (graph mean), odd rows 1/vcount
        vcrp = []
        for half, P in ((0, pa), (1, pb)):
            pm = ps_sm.tile([128, 1], F32, tag="sm")
            nc.tensor.matmul(out=pm[:], lhsT=P[:], rhs=vcrb[:], start=True, stop=True)
            vp = cp.tile([128, 1], F32, name=f"vcrp{half}", tag=f"vcrp{half}")
            nc.vector.tensor_add(out=vp[:], in0=pm[:], in1=chf[:])
            vcrp.append(vp)

        # ---------- first-node bookkeeping + gather indices (f32 exact) ----------
        curf = cp.tile([BC, 1], F32, name="curf")
        nc.vector.tensor_copy(out=curf[:], in_=cur[:])
        prvf = cp.tile([BC, 1], F32, name="prvf")
        nc.vector.tensor_copy(out=prvf[:], in_=prv[:])
        fstf = cp.tile([BC, 1], F32, name="fstf")
        nc.vector.tensor_copy(out=fstf[:], in_=fst[:])
        t1 = cp.tile([BC, 1], F32, name="t1")
        nc.vector.tensor_single_scalar(out=t1[:], in_=prvf[:], scalar=0.0, op=Alu.is_equal)
        t2 = cp.tile([BC, 1], F32, name="t2")
        nc.vector.tensor_single_scalar(out=t2[:], in_=curf[:], scalar=0.0, op=Alu.not_equal)
        ld = cp.tile([BC, 1], F32, name="ld")
        nc.vector.tensor_mul(out=ld[:], in0=t1[:], in1=t2[:])
        dd = cp.tile([BC, 1], F32, name="dd")
        nc.vector.tensor_sub(out=dd[:], in0=curf[:], in1=fstf[:])
        nc.vector.tensor_mul(out=dd[:], in0=ld[:], in1=dd[:])
        fnf = cp.tile([BC, 1], F32, name="fnf")
        nc.vector.tensor_add(out=fnf[:], in0=fstf[:], in1=dd[:])
        nc.vector.tensor_mul(out=fnf[:], in0=fnf[:], in1=t2[:])

        gcf = cp.tile([BC, 1], F32, name="gcf")
        nc.vector.tensor_add(out=gcf[:], in0=io5[:], in1=curf[:])
        gcur = cp.tile([BC, 1], I32, name="gcur")
        nc.vector.tensor_copy(out=gcur[:], in_=gcf[:])
        gff = cp.tile([BC, 1], F32, name="gff")
        nc.vector.tensor_add(out=gff[:], in0=io5[:], in1=fnf[:])
        gfn = cp.tile([BC, 1], I32, name="gfn")
        nc.vector.tensor_copy(out=gfn[:], in_=gff[:])

        # ---------- gathers ----------
        hc_rows = cp.tile([BC, D], BF16, name="hc_rows")
        nc.gpsimd.indirect_dma_start(
            out=hc_rows[:], out_offset=None, in_=emb_flat,
            in_offset=bass.IndirectOffsetOnAxis(ap=gcur[:, :1], axis=0))
        hf_rows = cp.tile([BC, D], BF16, name="hf_rows")
        nc.gpsimd.indirect_dma_start(
            out=hf_rows[:], out_offset=None, in_=emb_flat,
            in_offset=bass.IndirectOffsetOnAxis(ap=gfn[:, :1], axis=0))
        # indirect DMA sources must sit at offset 0: view the whole blob as
        # rows of the target width and fold the field's base row into the
        # index (all values < 2^24, exact in f32).
        h3_rows = blob.bitcast(U16).rearrange("(r c) -> r c", c=1)
        h3_base = float(_OFFS["h3u"][0] // 2)
        gh3f = cp.tile([BC, 1], F32, name="gh3f")
        nc.vector.tensor_scalar_add(out=gh3f[:], in0=gcf[:], scalar1=h3_base)
        gh3 = cp.tile([BC, 1], I32, name="gh3")
        nc.vector.tensor_copy(out=gh3[:], in_=gh3f[:])
        ch3u = cp.tile([BC, 1], U16, name="ch3u")
        nc.gpsimd.indirect_dma_start(
            out=ch3u[:], out_offset=None, in_=h3_rows,
            in_offset=bass.IndirectOffsetOnAxis(ap=gh3[:, :1], axis=0))
        ttm_rows = blob.bitcast(BF16).rearrange("(r c) -> r c", c=N)
        ttm_base = float(_OFFS["ttm"][0] // (2 * N))
        gtt = cp.tile([BC, 1], F32, name="gtt")
        nc.vector.tensor_copy(out=gtt[:], in_=ch3u[:])
        nc.vector.tensor_scalar_add(out=gtt[:], in0=gtt[:], scalar1=ttm_base)
        ch3 = cp.tile([BC, 1], I32, name="ch3")
        nc.vector.tensor_copy(out=ch3[:], in_=gtt[:])
        rbf = cp.tile([BC, N], BF16, name="rbf")
        nc.gpsimd.indirect_dma_start(
            out=rbf[:], out_offset=None, in_=ttm_rows,
            in_offset=bass.IndirectOffsetOnAxis(ap=ch3[:, :1], axis=0))
        # pre-scale by C_TRAVEL in place (s_sel carries 1.0 in fp8)
        nc.vector.tensor_scalar_mul(out=rbf[:], in0=rbf[:], scalar1=C_TRAVEL)

        # ---------- travel: T[cur_h3[b], h3[b,:]] * C_TRAVEL, 8 batches/call ----------
        trav = cp.tile([BC, N], F32, name="trav")
        for k in range(16):
            pt = ps_tr.tile([128, N], F32, tag="trep")
            nc.tensor.matmul(out=pt[:], lhsT=ssel[k][:], rhs=rbf[:], start=True, stop=True)
            sck = wk.tile([128, N], F32, tag="travsc")
            nc.vector.tensor_copy(out=sck[:], in_=pt[:])
            gk = wk.tile([128, N], F32, tag="travg")
            nc.gpsimd.indirect_copy(out=gk[:], data=sck[:], idxs=h3w[k][:],
                                    i_know_ap_gather_is_preferred=True)
            for g in range(8):  # extract batch row 16g+k (partition strides are
                r0 = 16 * g + k  # not AP-expressible; 8 tiny row DMAs instead)
                nc.sync.dma_start(out=trav[r0 : r0 + 1, :], in_=gk[r0 : r0 + 1, :])

        # ---------- W2 stationaries for the sums pass ----------
        # per n-tile t: [128, 4096]; batch b owns cols [32b, 32b+32):
        #   col 32b+2s   = 1.0   (s = b%16)  -> graph sum row
        #   col 32b+2s+1 = vf_b  -> visited sum row
        w2 = []
        for t in range(4):
            w = cp.tile([128, 4096], BF16, name=f"w2_{t}", tag=f"w2_{t}")
            nc.vector.memset(w[:], 0.0)
            ones_ap = w[:].rearrange("p (u c) -> p u c", u=8)[:, :, 0:512:34]
            nc.vector.memset(ones_ap, 1.0)
            vf_ap = w[:].rearrange("p (u c) -> p u c", u=8)[:, :, 1:512:34]
            nc.vector.tensor_copy(out=vf_ap, in_=visT[t][:].rearrange("p (u s) -> p u s", u=8))
            w2.append(w)

        # ---------- single stream: resident [d,n] chunks + transpose-back sums ----
        # chunk k covers batches 8k..8k+8 as [128 d, (j n)]; all 16 chunks stay
        # resident for the later score pass.  For the sums, the PE transposes
        # each [128,128] block back to natural [n%128, d] (8 blocks packed per
        # PSUM bank), DVE/Act alternately copy the bank to SBUF, and the w2
        # matmuls accumulate from there.
        psA = ps_sum.tile([128, D], F32, tag="sumA")
        psB = ps_sum.tile([128, D], F32, tag="sumB")
        ets = []
        for k in range(NCH):
            et = st.tile([128, 4096], BF16, tag="et", bufs=NCH)
            nc.sync.dma_start_transpose(et[:], emb_flat[4096 * k : 4096 * (k + 1), :])
            ets.append(et)
            for g in range(4):
                ptb = ps_tb.tile([128, 1024], BF16, tag="tb", bufs=2)
                for u in range(8):
                    idx = 8 * g + u
                    nc.tensor.matmul(
                        out=ptb[:, 128 * u : 128 * (u + 1)],
                        lhsT=et[:, 128 * idx : 128 * (idx + 1)], rhs=idn[:],
                        is_transpose=True, skip_group_check=True)
                natc = natp.tile([128, 1024], BF16, tag="natc", bufs=2)
                if g % 2 == 0:
                    nc.vector.tensor_copy(out=natc[:], in_=ptb[:])
                else:
                    nc.scalar.copy(out=natc[:], in_=ptb[:])
                for u in range(8):
                    idx = 8 * g + u
                    j, t = divmod(idx, 4)
                    b = k * CHB + j
                    half, r = b // 64, b % 64
                    jj, s = r // 16, r % 16
                    ps = psA if half == 0 else psB
                    nc.tensor.matmul(
                        out=ps[32 * jj : 32 * jj + 32, :],
                        lhsT=w2[t][:, 32 * b : 32 * b + 32],
                        rhs=natc[:, 128 * u : 128 * (u + 1)],
                        start=(s == 0 and t == 0), stop=True,
                        tile_position=(0, 32 * jj), skip_group_check=True)

        # ---------- sums -> G^T / V^T (dense, bf16, [128e, 128b]) ----------
        gt = cp.tile([128, BC], BF16, name="gt")
        vt = cp.tile([128, BC], BF16, name="vt")
        for half, ps in ((0, psA), (1, psB)):
            gvr = wk.tile([128, 128], BF16, tag="gvr")
            nc.vector.tensor_scalar(out=gvr[:], in0=ps[:], scalar1=vcrp[half][:, :1],
                                    scalar2=None, op0=Alu.mult)
            pt = ps_sm.tile([128, 128], BF16, tag="gvt", bufs=1)
            nc.tensor.transpose(out=pt[:], in_=gvr[:], identity=idn[:])
            # cols m=32j+2s -> batch 64*half+16j+s
            src_g = pt[:].rearrange("p (j c) -> p j c", j=4)[:, :, 0:32:2]
            src_v = pt[:].rearrange("p (j c) -> p j c", j=4)[:, :, 1:32:2]
            dst_g = gt[:, 64 * half : 64 * half + 64].rearrange("p (j s) -> p j s", j=4)
            dst_v = vt[:, 64 * half : 64 * half + 64].rearrange("p (j s) -> p j s", j=4)
            nc.vector.tensor_copy(out=dst_g, in_=src_g)
            nc.vector.tensor_copy(out=dst_v, in_=src_v)

        # ---------- h_cur/h_first transposes ----------
        hct = cp.tile([128, BC], BF16, name="hct")
        pt1 = ps_sm.tile([128, 128], BF16, tag="gvt", bufs=1)
        nc.tensor.transpose(out=pt1[:], in_=hc_rows[:], identity=idn[:])
        nc.vector.tensor_copy(out=hct[:], in_=pt1[:])
        hft = cp.tile([128, BC], BF16, name="hft")
        pt2 = ps_sm.tile([128, 128], BF16, tag="gvt", bufs=1)
        nc.tensor.transpose(out=pt2[:], in_=hf_rows[:], identity=idn[:])
        nc.vector.tensor_copy(out=hft[:], in_=pt2[:])

        # ---------- state feats ----------
        sf = cp.tile([BC, 3], F32, name="sf")
        nc.vector.tensor_sub(out=sf[:, 0:1], in0=sc4[:, 2:3], in1=sc4[:, 1:2])
        nc.vector.tensor_scalar_mul(out=sf[:, 1:2], in0=sc4[:, 0:1], scalar1=1.0 / MAX_TIME)
        nc.vector.tensor_scalar_mul(out=sf[:, 2:3], in0=sc4[:, 3:4], scalar1=1.0 / (2.0 * N))
        sfb = cp.tile([BC, 3], BF16, name="sfb")
        nc.vector.tensor_copy(out=sfb[:], in_=sf[:])
        pt3 = ps_sm.tile([128, 128], BF16, tag="gvt", bufs=1)
        nc.tensor.transpose(out=pt3[:3, :], in_=sfb[:], identity=idn[:])
        sft = cp.tile([3, BC], BF16, name="sft")
        nc.vector.tensor_copy(out=sft[:], in_=pt3[:3, :BC])

        # ---------- q^T and qk^T ----------
        psq = ps_sm.tile([128, BC], F32, tag="sm")
        nc.tensor.matmul(out=psq[:], lhsT=wl[:], rhs=hct[:], start=True, stop=True)
        nc.tensor.matmul(out=psq[:], lhsT=wf[:], rhs=hft[:], start=False, stop=True,
                         skip_group_check=True)
        nc.tensor.matmul(out=psq[:], lhsT=wg[:], rhs=gt[:], start=False, stop=True,
                         skip_group_check=True)
        nc.tensor.matmul(out=psq[:], lhsT=wv[:], rhs=vt[:], start=False, stop=True,
                         skip_group_check=True)
        nc.tensor.matmul(out=psq[:], lhsT=ws[:], rhs=sft[:], start=False, stop=True,
                         skip_group_check=True)
        qt = cp.tile([128, BC], BF16, name="qt")
        nc.vector.tensor_scalar(out=qt[:], in0=psq[:], scalar1=bst[:, :1], scalar2=None,
                                op0=Alu.add)
        psk = ps_sm.tile([128, BC], F32, tag="sm")
        nc.tensor.matmul(out=psk[:], lhsT=wkT[:], rhs=qt[:], start=True, stop=True)
        qk = cp.tile([128, BC], BF16, name="qk")
        nc.vector.tensor_scalar_mul(out=qk[:], in0=psk[:], scalar1=INV_SQRT_D)

        # ---------- qkpad: batch b -> col 32b + (b%32) ----------
        qkp = cp.tile([128, 4096], BF16, name="qkp")
        nc.vector.memset(qkp[:], 0.0)
        for J in range(4):
            # batch b = 32J + r -> col 32b + r = 1024J + 33r (out row = 32J + r = b)
            nc.vector.tensor_copy(out=qkp[:, 1024 * J : 1024 * (J + 1) : 33],
                                  in_=qk[:, 32 * J : 32 * (J + 1)])

        # ---------- score pass: re-read the resident chunks -> psum [128b, 512n] ----
        pssc = ps_sc.tile([128, N], F32, tag="score")
        for k in range(NCH):
            for j in range(CHB):
                b = k * CHB + j
                J, r = b // 32, b % 32
                nc.tensor.matmul(
                    out=pssc[32 * J : 32 * J + 32, :],
                    lhsT=qkp[:, 32 * b : 32 * b + 32],
                    rhs=ets[k][:, j * N : (j + 1) * N],
                    start=(r == 0), stop=True,
                    tile_position=(0, 32 * J), skip_group_check=True)

        # ---------- epilogue: travel, tanh, mask, log_softmax ----------
        # msk = 10*amf*tanh(s/10) + (amf-1)*1e8 = amf*(10*tanh(s/10) + 1e8) - 1e8
        # computed in 3 scratch tiles (eA/eB/eC) with in-place updates.
        if DEBUG_TAPS:
            for nm, tl in [("d_gt", gt), ("d_vt", vt),
                           ("d_hct", hct), ("d_hft", hft), ("d_qt", qt), ("d_qk", qk)]:
                tmpd = wk.tile([tl.shape[0], tl.shape[1]], F32, tag=f"tap{nm}", bufs=1)
                nc.vector.tensor_copy(out=tmpd[:], in_=tl[:])
                nc.sync.dma_start(out=ap[nm], in_=tmpd[:])
        eA = wk.tile([BC, N], F32, tag="eA", bufs=1)
        nc.vector.tensor_sub(out=eA[:], in0=pssc[:], in1=trav[:])
        if DEBUG_TAPS:
            nc.sync.dma_start(out=ap["d_score"], in_=eA[:])
        eB = wk.tile([BC, N], F32, tag="eB", bufs=1)
        nc.scalar.activation(out=eB[:], in_=eA[:], func=AF.Tanh, scale=1.0 / TANH_CLIP)
        # msk = tanh*(10*amf) + (amf-1)*1e8 — keep the 1e8 term separate so the
        # small tanh term is never absorbed into 1e8's 8-ulp grid.
        eC = wk.tile([BC, N], F32, tag="eC", bufs=1)
        nc.vector.tensor_scalar_mul(out=eC[:], in0=amf[:], scalar1=TANH_CLIP)
        nc.vector.tensor_mul(out=eC[:], in0=eB[:], in1=eC[:])
        nc.vector.tensor_scalar(out=eA[:], in0=amf[:], scalar1=1.0, scalar2=1e8,
                                op0=Alu.subtract, op1=Alu.mult)
        nc.vector.tensor_add(out=eA[:], in0=eC[:], in1=eA[:])
        if DEBUG_TAPS:
            nc.sync.dma_start(out=ap["d_msk"], in_=eA[:])

        mx = cp.tile([BC, 1], F32, name="mx")
        nc.vector.tensor_reduce(out=mx[:], in_=eA[:], axis=AX.X, op=Alu.max)
        ngm = cp.tile([BC, 1], F32, name="ngm")
        nc.vector.tensor_scalar_mul(out=ngm[:], in0=mx[:], scalar1=-1.0)
        se = cp.tile([BC, 1], F32, name="se")
        nc.scalar.activation(out=eB[:], in_=eA[:], func=AF.Exp, bias=ngm[:, :1],
                             scale=1.0, accum_out=se[:])
        lse = cp.tile([BC, 1], F32, name="lse")
        nc.scalar.activation(out=lse[:], in_=se[:], func=AF.Ln)
        nc.vector.tensor_scalar(out=eA[:], in0=eA[:], scalar1=mx[:, :1], scalar2=None,
                                op0=Alu.subtract)
        nc.vector.tensor_scalar(out=eA[:], in0=eA[:], scalar1=lse[:, :1], scalar2=None,
                                op0=Alu.subtract)
        nc.sync.dma_start(out=ap["out"], in_=eA[:])


def build_program():
    nc = bacc.Bacc("TRN2", target_bir_lowering=False, debug=False)
    dt = nc.dram_tensor
    T = {}
    T["blob"] = dt("blob", [BLOB_BYTES], U8, kind="ExternalInput")
    T["out"] = dt("out", [BC, N], F32, kind="ExternalOutput")
    if DEBUG_TAPS:
        for nm, shp in [("d_gt", [128, BC]), ("d_vt", [128, BC]),
                        ("d_hct", [128, BC]), ("d_hft", [128, BC]), ("d_qt", [128, BC]),
                        ("d_qk", [128, BC]), ("d_score", [BC, N]), ("d_msk", [BC, N])]:
            T[nm] = dt(nm, shp, F32, kind="ExternalOutput")

    with tile.TileContext(nc) as tc:
        _emit(nc, tc, T)
    nc.compile()
    return nc


@functools.cache
def _cached_program():
    return build_program()


def _consts():
    c = {}
    c["ident"] = np.eye(128, dtype=NBF)
    s = np.zeros((16, 128, 128), dtype=mybir.dt.np(FP8))
    pidx = np.arange(128)
    for k in range(16):
        s[k, (pidx // 16) * 16 + k, pidx] = 1.0
    c["s_sel"] = s
    pa = np.zeros((128, 128), dtype=NBF)
    pb = np.zeros((128, 128), dtype=NBF)
    for b in range(64):
        m = 32 * (b // 16) + 2 * (b % 16) + 1
        pa[b, m] = 1
        pb[64 + b, m] = 1
    c["p_a"], c["p_b"] = pa, pb
    ch = np.zeros((128, 1), np.float32)
    ch[0::2] = 1.0 / N
    c["c_half"] = ch
    c["iota512f"] = (np.arange(BC, dtype=np.float32) * N)[:, None]
    return c


def make_in_map(inputs, core, consts=None):
    """Host-side shard + pack into the per-core blob (pure layout/dtype work)."""
    sl = slice(BC * core, BC * (core + 1))
    blob = np.zeros(BLOB_BYTES, np.uint8)

    def fv(name):
        off, dt_, shape, nb = _OFFS[name]
        return blob[off : off + nb].view(mybir.dt.np(dt_)).reshape(shape)

    fv("emb")[:] = np.asarray(inputs["node_emb"][sl], dtype=np.float32)
    h3 = np.asarray(inputs["h3_indices"][sl]).astype(np.uint16)  # [128, 512]
    fv("h3u")[:] = h3.reshape(BC * N, 1)
    fv("h3_wrap")[:] = np.ascontiguousarray(
        h3.reshape(8, 16, 32, 16).transpose(1, 0, 3, 2)).reshape(16, 128, 32)
    fv("ttm")[:] = np.asarray(inputs["travel_time_matrix"], dtype=np.float32)
    vis = np.asarray(inputs["visited"][sl]).astype(np.uint8)
    fv("vis_rows")[:] = vis
    fv("visT_bf")[:] = vis.T
    fv("am_rows")[:] = np.asarray(inputs["action_mask"][sl]).astype(np.uint8)
    fv("w_last")[:] = np.asarray(inputs["W_last"], np.float32)
    fv("w_first")[:] = np.asarray(inputs["W_first"], np.float32)
    fv("w_graph")[:] = np.asarray(inputs["W_graph"], np.float32)
    fv("w_visited")[:] = np.asarray(inputs["W_visited"], np.float32)
    fv("w_keyT")[:] = np.asarray(inputs["W_key"], np.float32).T
    fv("w_state")[:] = np.asarray(inputs["W_state"], np.float32)
    fv("b_state")[:] = np.asarray(inputs["b_state"], np.float32).reshape(D, 1)
    fv("scal4")[:] = np.concatenate(
        [np.asarray(inputs["current_time"][sl], np.float32),
         np.asarray(inputs["used_capacity"][sl], np.float32),
         np.asarray(inputs["vehicle_capacity"][sl], np.float32),
         np.asarray(inputs["i"][sl]).astype(np.float32)], axis=1)
    fv("cur_i")[:] = np.asarray(inputs["current_node"][sl]).astype(np.int32).reshape(BC, 1)
    fv("prev_i")[:] = np.asarray(inputs["previous_action"][sl]).astype(np.int32).reshape(BC, 1)
    fv("first_i")[:] = np.asarray(inputs["first_node"][sl]).astype(np.int32).reshape(BC, 1)
    for k, v in (consts or _consts()).items():
        fv(k)[:] = v
    return {"blob": blob}


_last_results = None


def kernel(**inputs):
    global _last_results
    from concurrent.futures import ThreadPoolExecutor
    nc = _cached_program()
    consts = _consts()
    inputs = {k: np.asarray(v) for k, v in inputs.items()}
    with ThreadPoolExecutor(NCORES) as ex:
        in_maps = list(ex.map(lambda c: make_in_map(inputs, c, consts),
                              range(NCORES)))
    trace = bool(int(os.environ.get("KERNEL_TRACE", "0")))
    rr = run_bass_kernel_spmd(nc, in_maps, list(range(NCORES)), trace=trace)
    _last_results = rr
    out = np.concatenate([np.asarray(rr.results[c]["out"], np.float32)
                          for c in range(NCORES)], axis=0)
    return out
